# revision 1
# baseline (speedup 1.0000x reference)
"""Segment mean-pooling (scatter_mean) on 8 Trainium2 NeuronCores.

Strategy (data-parallel rows, per the sharding hint):
  - Host shards the 4M rows of x/index across the 8 cores (500K rows
    each), padding each shard to 62*8192 rows (pad rows route to a
    dump bucket that is never read back).
  - Kernel 1 (bucketize), per core: for each 128-row tile, compute
    each row's bucket (idx >> 9; 196 buckets x 512 segments) and its
    exact rank within the bucket via a strict-lower-triangular matmul
    prefix-count on the tensor engine plus a running per-bucket base
    vector; assemble 34-wide rows [x(32) | 1.0 | idx] and scatter each
    tile with a [128,1]-index indirect DMA into bucket-contiguous
    staging (slots are unique by construction - race-free).
  - Kernel 2 (accumulate), per core: for each bucket, bulk-load its
    staged rows, build a 512-wide one-hot from the stored idx on the
    vector engine, and matmul-accumulate [33, 512] PSUM tiles (32
    feature sums + count), writing a transposed partial table
    [33, 196*512].
  - Host all-reduces the 8 partial tables (sum), divides sums by
    max(count, 1), and transposes to the final [100000, 32] output.
"""
import numpy as np
import concourse.bass as bass
import concourse.bacc as bacc
import concourse.tile as tile
import concourse.mybir as mybir
from concourse.bass_utils import run_bass_kernel_spmd

F32 = mybir.dt.float32
I32 = mybir.dt.int32
OP = mybir.AluOpType

N_ROWS = 4000000
D = 32
NUM_SEGMENTS = 100000
N_CORES = 8
N_CHUNKS = 62          # per-core row chunks of 8192 (62*8192 = 507904)
E = 34                 # staged row: x(32) | 1.0 | idx
SEG_PER_B = 512        # segments per bucket (bucket = idx >> 9)
NB = 196               # normal buckets (196*512 = 100352 >= 100000)
CAP = 3072             # slots per bucket (mean 2560 + ~10 sigma)
DUMP_CAP = 8192        # slots for the padding dump bucket
BW = 200               # bucket one-hot width (padded)
AX_X = mybir.AxisListType.X

_cache = {}


def _k1_build():
    slots_total = NB * CAP + DUMP_CAP
    R = N_CHUNKS * 8192
    nc = bacc.Bacc("TRN2", target_bir_lowering=False, debug=False,
                   num_devices=N_CORES)
    x_d = nc.dram_tensor("x", [R, D], F32, kind="ExternalInput")
    i_d = nc.dram_tensor("idx", [R], I32, kind="ExternalInput")
    tri_d = nc.dram_tensor("tri", [128, 128], F32, kind="ExternalInput")
    ones_d = nc.dram_tensor("ones1", [1, 128], F32, kind="ExternalInput")
    onesc_d = nc.dram_tensor("onescol", [128, 1], F32, kind="ExternalInput")
    iota_d = nc.dram_tensor("iotab", [128, BW], F32, kind="ExternalInput")
    start_d = nc.dram_tensor("startv", [1, BW], F32, kind="ExternalInput")
    stage_d = nc.dram_tensor("staging", [slots_total, E], F32,
                             kind="ExternalOutput")
    with tile.TileContext(nc) as tc:
        with tc.tile_pool(name="const", bufs=1) as cp, \
             tc.tile_pool(name="sbuf", bufs=3) as pool, \
             tc.tile_pool(name="psum", bufs=4, space="PSUM") as pp:
            tri = cp.tile([128, 128], F32)
            nc.sync.dma_start(out=tri[:], in_=tri_d.ap())
            ones1 = cp.tile([1, 128], F32)
            nc.sync.dma_start(out=ones1[:], in_=ones_d.ap())
            onescol = cp.tile([128, 1], F32)
            nc.sync.dma_start(out=onescol[:], in_=onesc_d.ap())
            iota = cp.tile([128, BW], F32)
            nc.sync.dma_start(out=iota[:], in_=iota_d.ap())
            base = cp.tile([1, BW], F32)
            nc.sync.dma_start(out=base[:], in_=start_d.ap())
            for c in range(N_CHUNKS):
                r0 = c * 8192
                xt = pool.tile([128, 64 * D], F32, tag="x")
                nc.sync.dma_start(
                    out=xt[:],
                    in_=x_d.ap()[r0:r0 + 8192, :].rearrange(
                        "(p k) d -> p (k d)", p=128))
                iti = pool.tile([128, 64], I32, tag="ii")
                nc.sync.dma_start(
                    out=iti[:],
                    in_=i_d.ap()[r0:r0 + 8192].rearrange("(p k) -> p k", p=128))
                bbi = pool.tile([128, 64], I32, tag="bi")
                nc.vector.tensor_scalar(out=bbi[:], in0=iti[:], scalar1=9,
                                        scalar2=None, op0=OP.arith_shift_right)
                bbf = pool.tile([128, 64], F32, tag="bf")
                nc.vector.tensor_copy(out=bbf[:], in_=bbi[:])
                idxf = pool.tile([128, 64], F32, tag="if")
                nc.vector.tensor_copy(out=idxf[:], in_=iti[:])
                at = pool.tile([128, 64 * E], F32, tag="at")
                at3 = at[:].rearrange("p (k e) -> p k e", e=E)
                nc.vector.memset(at3[:, :, D:D + 1], 1.0)
                nc.vector.tensor_copy(
                    out=at3[:, :, 0:D],
                    in_=xt[:].rearrange("p (k d) -> p k d", d=D))
                nc.vector.tensor_copy(out=at3[:, :, D + 1:E],
                                      in_=idxf[:].unsqueeze(-1))
                slots_f = pool.tile([128, 64], F32, tag="sf")
                slots_i = pool.tile([128, 64], I32, tag="si")
                for t in range(64):
                    M = pool.tile([128, BW], F32, tag="M")
                    nc.vector.tensor_tensor(
                        out=M[:],
                        in0=bbf[:, t:t + 1].to_broadcast([128, BW]),
                        in1=iota[:], op=OP.is_equal)
                    cum = pp.tile([128, BW], F32, space="PSUM", tag="cum")
                    nc.tensor.matmul(out=cum[:], lhsT=ones1[:], rhs=base[:],
                                     start=True, stop=False)
                    nc.tensor.matmul(out=cum[:], lhsT=tri[:], rhs=M[:],
                                     start=False, stop=True)
                    scr = pool.tile([128, BW], F32, tag="scr")
                    nc.vector.tensor_tensor(out=scr[:], in0=cum[:],
                                            in1=M[:], op=OP.mult)
                    nc.vector.tensor_reduce(out=slots_f[:, t:t + 1],
                                            in_=scr[:], axis=AX_X, op=OP.add)
                    csum = pp.tile([1, BW], F32, space="PSUM", tag="csum")
                    nc.tensor.matmul(out=csum[:], lhsT=onescol[:], rhs=M[:],
                                     start=True, stop=True)
                    nc.vector.tensor_tensor(out=base[:], in0=csum[:],
                                            in1=base[:], op=OP.add)
                    nc.vector.tensor_copy(out=slots_i[:, t:t + 1],
                                          in_=slots_f[:, t:t + 1])
                    nc.gpsimd.indirect_dma_start(
                        out=stage_d.ap(),
                        out_offset=bass.IndirectOffsetOnAxis(
                            ap=slots_i[:, t:t + 1], axis=0),
                        in_=at[:, t * E:(t + 1) * E],
                        in_offset=None)
    nc.compile()
    return nc


def _k2_build():
    slots_total = NB * CAP + DUMP_CAP
    TPB = CAP // 128
    nc = bacc.Bacc("TRN2", target_bir_lowering=False, debug=False,
                   num_devices=N_CORES)
    stage_d = nc.dram_tensor("staging", [slots_total, E], F32,
                             kind="ExternalInput")
    iota_d = nc.dram_tensor("iota512", [128, SEG_PER_B], F32,
                            kind="ExternalInput")
    out_d = nc.dram_tensor("tableT", [D + 1, NB * SEG_PER_B], F32,
                           kind="ExternalOutput")
    with tile.TileContext(nc) as tc:
        with tc.tile_pool(name="const", bufs=1) as cp, \
             tc.tile_pool(name="sbuf", bufs=3) as pool, \
             tc.tile_pool(name="psum", bufs=2, space="PSUM") as pp:
            iota = cp.tile([128, SEG_PER_B], F32)
            nc.sync.dma_start(out=iota[:], in_=iota_d.ap())
            for b in range(NB):
                st = pool.tile([128, TPB * E], F32, tag="st")
                nc.sync.dma_start(
                    out=st[:],
                    in_=stage_d.ap()[b * CAP:(b + 1) * CAP, :].rearrange(
                        "(p r) e -> p (r e)", p=128))
                iob = pool.tile([128, SEG_PER_B], F32, tag="iob")
                nc.vector.tensor_scalar(out=iob[:], in0=iota[:],
                                        scalar1=float(b * SEG_PER_B),
                                        scalar2=None, op0=OP.add)
                ps = pp.tile([D + 1, SEG_PER_B], F32, space="PSUM", tag="ps")
                for t in range(TPB):
                    oh = pool.tile([128, SEG_PER_B], F32, tag="oh")
                    nc.vector.tensor_tensor(
                        out=oh[:],
                        in0=st[:, t * E + D + 1:t * E + E].to_broadcast(
                            [128, SEG_PER_B]),
                        in1=iob[:], op=OP.is_equal)
                    nc.tensor.matmul(out=ps[:], lhsT=st[:, t * E:t * E + D + 1],
                                     rhs=oh[:], start=(t == 0),
                                     stop=(t == TPB - 1))
                ob = pool.tile([D + 1, SEG_PER_B], F32, tag="ob")
                nc.vector.tensor_copy(out=ob[:], in_=ps[:])
                nc.sync.dma_start(
                    out=out_d.ap()[:, b * SEG_PER_B:(b + 1) * SEG_PER_B],
                    in_=ob[:])
    nc.compile()
    return nc


def _consts():
    tri = (np.arange(128)[:, None] < np.arange(128)[None, :]).astype(np.float32)
    ones1 = np.ones((1, 128), np.float32)
    onescol = np.ones((128, 1), np.float32)
    iotab = np.tile(np.arange(BW, dtype=np.float32), (128, 1))
    startv = np.zeros((1, BW), np.float32)
    for b in range(NB):
        startv[0, b] = b * CAP
    for b in range(NB, BW):
        startv[0, b] = NB * CAP  # dump bucket (and unused tail)
    iota512 = np.tile(np.arange(SEG_PER_B, dtype=np.float32), (128, 1))
    return tri, ones1, onescol, iotab, startv, iota512


def kernel(x, index):
    x = np.ascontiguousarray(np.asarray(x, dtype=np.float32))
    idx = np.asarray(index)
    assert x.shape == (N_ROWS, D)
    if "k1" not in _cache:
        _cache["k1"] = _k1_build()
        _cache["k2"] = _k2_build()
    k1, k2 = _cache["k1"], _cache["k2"]
    tri, ones1, onescol, iotab, startv, iota512 = _consts()
    idx32 = idx.astype(np.int32)
    per = N_ROWS // N_CORES
    R = N_CHUNKS * 8192
    for c in range(N_CORES):
        bc = np.bincount(idx32[c * per:(c + 1) * per] >> 9, minlength=NB)
        if bc.max() > CAP:
            raise RuntimeError(
                f"bucket overflow on core {c}: {bc.max()} > {CAP} rows in one "
                f"512-segment bucket (kernel sized for uniform indices)")
    in1 = []
    for c in range(N_CORES):
        xs = np.zeros((R, D), np.float32)
        xs[:per] = x[c * per:(c + 1) * per]
        ii = np.full((R,), NB * SEG_PER_B, np.int32)  # pad -> dump bucket
        ii[:per] = idx32[c * per:(c + 1) * per]
        in1.append({"x": xs, "idx": ii, "tri": tri, "ones1": ones1,
                    "onescol": onescol, "iotab": iotab, "startv": startv})
    r1 = run_bass_kernel_spmd(k1, in1, list(range(N_CORES))).results
    in2 = [{"staging": r1[c]["staging"], "iota512": iota512}
           for c in range(N_CORES)]
    r2 = run_bass_kernel_spmd(k2, in2, list(range(N_CORES))).results
    acc = np.zeros((D + 1, NB * SEG_PER_B), np.float64)
    for c in range(N_CORES):
        acc += r2[c]["tableT"]
    sums = acc[:D, :NUM_SEGMENTS].T
    counts = acc[D, :NUM_SEGMENTS]
    out = sums / np.maximum(counts, 1.0)[:, None]
    return out.astype(np.float32)



# revision 4
# speedup vs baseline: 13.7488x; 13.7488x over previous
"""Segment mean-pooling (scatter_mean) on 8 Trainium2 NeuronCores.

Strategy (segment-sharded, host-routed):
  - The output segment range [0, 100352) is sharded across the 8 cores
    (12544 segments each), so each core produces a disjoint slice of the
    output table and no all-reduce is needed.
  - The host stable-sorts rows by segment id (this is the shard/routing
    step: each row is sent to the core that owns its segment), casts x
    to bf16, appends a ones column (for counts), and packs each core's
    rows into per-bucket tiles of 128 rows. A bucket is a 64-segment
    window; tile counts per bucket position are maxed across cores so a
    single SPMD program serves all 8 cores.
  - Device, per core: stream the packed rows contiguously (no indirect
    DMA), build a [128, 64] bf16 one-hot per tile from the per-row
    segment-lo values (tensor_scalar is_equal, split across the DVE and
    Pool engines), and matmul-accumulate [34, 64] blocks into a shared
    [34, 512] PSUM group (8 buckets per PSUM bank). The Act engine
    evacuates finished PSUM groups into an SBUF-resident output table
    [34, 12544] which is written back in one bulk DMA.
  - Host: concatenate the 8 disjoint slices, divide sums by
    max(count, 1), transpose to [100000, 32].
"""
import numpy as np
import ml_dtypes
import concourse.bacc as bacc
import concourse.tile as tile
import concourse.mybir as mybir
from concourse.bass_utils import run_bass_kernel_spmd

F32 = mybir.dt.float32
BF16 = mybir.dt.bfloat16
OP = mybir.AluOpType
ACT_COPY = mybir.ActivationFunctionType.Copy

N_ROWS = 4000000
D = 32
NUM_SEGMENTS = 100000
N_CORES = 8
W = 64                 # segments per bucket (one-hot width)
GB = 8                 # buckets per PSUM group ([34, 512] = one 2KB bank)
E = 34                 # packed row: x(32) | 1.0 | pad
S_PAD = 100352         # 8 * 12544, >= NUM_SEGMENTS
SEG_PER_CORE = S_PAD // N_CORES      # 12544
NB = SEG_PER_CORE // W               # 196 buckets per core
N_GROUPS = NB // GB                  # 24.5 -> NB must be divisible; 196/8=24.5
CHUNK = 64             # tiles per xe load
DVE_SHARE = 0.69       # fraction of one-hot builds on DVE (rest on Pool)

_cache = {}


def _build(tiles):
    """Build the SPMD kernel for per-bucket tile counts `tiles` (len NB,
    every entry >= 1; identical across cores)."""
    total_tiles = sum(tiles)
    R = total_tiles * 128
    nc = bacc.Bacc("TRN2", target_bir_lowering=False, debug=False,
                   num_devices=N_CORES)
    xe_d = nc.dram_tensor("xe", [R * E], BF16, kind="ExternalInput")
    lo_d = nc.dram_tensor("lo", [R], F32, kind="ExternalInput")
    iota_d = nc.dram_tensor("iota", [128, W], BF16, kind="ExternalInput")
    out_d = nc.dram_tensor("tab", [E, SEG_PER_CORE], F32,
                           kind="ExternalOutput")
    groups = [list(range(g0, min(g0 + GB, NB))) for g0 in range(0, NB, GB)]
    with tile.TileContext(nc) as tc:
        with tc.tile_pool(name="const", bufs=1) as cp, \
             tc.tile_pool(name="stream", bufs=3) as pool, \
             tc.tile_pool(name="ohp", bufs=8) as ohpool, \
             tc.tile_pool(name="psum", bufs=4, space="PSUM") as pp:
            iota = cp.tile([128, W], BF16)
            nc.sync.dma_start(out=iota[:], in_=iota_d.ap())
            ost = cp.tile([E, SEG_PER_CORE], F32)
            g_base = 0     # running tile offset
            acc = 0.0      # DVE/Pool alternation accumulator
            for g, bks in enumerate(groups):
                gw = len(bks) * W
                Tg = sum(tiles[b] for b in bks)
                ps = pp.tile([E, GB * W], F32, space="PSUM", tag="ps")
                xe_g = xe_d.ap()[g_base * 128 * E:(g_base + Tg) * 128 * E] \
                    .rearrange("(p q) -> p q", p=128)
                lo_g = lo_d.ap()[g_base * 128:(g_base + Tg) * 128] \
                    .rearrange("(p q) -> p q", p=128)
                seq = [(b, i) for b in bks for i in range(tiles[b])]
                for c0 in range(0, len(seq), CHUNK):
                    sub = seq[c0:c0 + CHUNK]
                    nsub = len(sub)
                    xe = pool.tile([128, nsub * E], BF16, tag="xe")
                    nc.sync.dma_start(out=xe[:],
                                      in_=xe_g[:, c0 * E:(c0 + nsub) * E])
                    lof = pool.tile([128, nsub], F32, tag="lo")
                    nc.sync.dma_start(out=lof[:], in_=lo_g[:, c0:c0 + nsub])
                    for j, (b, i) in enumerate(sub):
                        oh = ohpool.tile([128, W], BF16, tag="oh")
                        acc += DVE_SHARE
                        if acc >= 1.0:
                            acc -= 1.0
                            eng = nc.vector
                        else:
                            eng = nc.gpsimd
                        eng.tensor_scalar(out=oh[:], in0=iota[:],
                                          scalar1=lof[:, j:j + 1],
                                          scalar2=None, op0=OP.is_equal)
                        cw = (b - bks[0]) * W
                        nc.tensor.matmul(out=ps[:, cw:cw + W],
                                         lhsT=xe[:, j * E:(j + 1) * E],
                                         rhs=oh[:], start=(i == 0),
                                         stop=(i == tiles[b] - 1))
                nc.scalar.activation(out=ost[:, g * GB * W:g * GB * W + gw],
                                     in_=ps[:, :gw], func=ACT_COPY)
                g_base += Tg
            nc.sync.dma_start(out=out_d.ap(), in_=ost[:])
    nc.compile()
    return nc


def _pack_core(xb_sorted, lo_sorted, gbkt_sorted, row0, row1, core, tiles):
    """Pack one core's sorted rows into group-major [128, Tg, E] blocks.

    xb_sorted: [N, E] bf16 rows (x | 1 | pad), sorted by segment id.
    lo_sorted: [N] f32 segment-lo (idx % W) per sorted row.
    gbkt_sorted: [N] int32 global bucket id (idx // W) per sorted row.
    Rows [row0, row1) belong to this core.
    """
    xeb = []
    lob = []
    bkt = gbkt_sorted[row0:row1] - core * NB
    # bucket start offsets within the core's row range
    starts = np.searchsorted(bkt, np.arange(NB + 1))
    for b in range(NB):
        Tb = tiles[b]
        a, z = row0 + starts[b], row0 + starts[b + 1]
        nb_rows = z - a
        xx = np.zeros((Tb * 128, E), ml_dtypes.bfloat16)
        xx[:nb_rows] = xb_sorted[a:z]
        ll = np.full(Tb * 128, -1.0, np.float32)
        ll[:nb_rows] = lo_sorted[a:z]
        xeb.append(np.ascontiguousarray(
            xx.reshape(Tb, 128, E).transpose(1, 0, 2)))
        lob.append(np.ascontiguousarray(ll.reshape(Tb, 128).T))
    xe_parts = []
    lo_parts = []
    for g0 in range(0, NB, GB):
        xe_parts.append(np.concatenate(xeb[g0:g0 + GB], axis=1).ravel())
        lo_parts.append(np.concatenate(lob[g0:g0 + GB], axis=1).ravel())
    return np.concatenate(xe_parts), np.concatenate(lo_parts)


def kernel(x, index):
    x = np.asarray(x, dtype=np.float32)
    idx = np.asarray(index).astype(np.int32)
    assert x.shape == (N_ROWS, D) and idx.shape == (N_ROWS,)

    # --- host routing: sort rows by segment, shard segment ranges ---
    order = np.argsort(idx, kind="stable")
    sidx = idx[order]
    gbkt = sidx // W                                   # global bucket id
    cnt = np.bincount(gbkt, minlength=N_CORES * NB).reshape(N_CORES, NB)
    tiles = np.maximum(((cnt + 127) // 128).max(axis=0), 1)  # max over cores
    tiles = [int(t) for t in tiles]

    key = tuple(tiles)
    if _cache.get("key") != key:
        _cache["nc"] = _build(tiles)
        _cache["key"] = key
    nc = _cache["nc"]

    xb = np.zeros((N_ROWS, E), ml_dtypes.bfloat16)
    xb[:, :D] = x[order].astype(ml_dtypes.bfloat16)
    xb[:, D] = 1.0
    lo = (sidx % W).astype(np.float32)
    bounds = np.searchsorted(sidx, np.arange(N_CORES + 1) * SEG_PER_CORE)
    iota = np.tile(np.arange(W, dtype=np.float32), (128, 1)) \
        .astype(ml_dtypes.bfloat16)

    in_maps = []
    for c in range(N_CORES):
        xe_c, lo_c = _pack_core(xb, lo, gbkt, bounds[c], bounds[c + 1],
                                c, tiles)
        in_maps.append({"xe": xe_c, "lo": lo_c, "iota": iota})

    res = run_bass_kernel_spmd(nc, in_maps, list(range(N_CORES))).results

    tab = np.concatenate([res[c]["tab"] for c in range(N_CORES)], axis=1)
    sums = tab[:D, :NUM_SEGMENTS].T.astype(np.float64)
    counts = tab[D, :NUM_SEGMENTS].astype(np.float64)
    out = sums / np.maximum(counts, 1.0)[:, None]
    return out.astype(np.float32)


# revision 8
# speedup vs baseline: 40.6237x; 2.9547x over previous
"""Segment mean-pooling (scatter_mean) on 8 Trainium2 NeuronCores.

Strategy (segment-sharded, host-routed):
  - The output segment range [0, 100352) is sharded across the 8 cores
    (12544 segments each), so each core produces a disjoint slice of the
    output table and no all-reduce is needed.
  - The host stable-sorts rows by segment id (this is the shard/routing
    step: each row is sent to the core that owns its segment), casts x
    to bf16, appends a ones column (for counts), and packs each core's
    rows into per-bucket tiles of 128 rows. A bucket is a 16-segment
    window; tile counts per bucket position are maxed across cores so a
    single SPMD program serves all 8 cores.
  - Device, per core: stream the packed rows contiguously (no indirect
    DMA), build bf16 one-hots in 16-tile batches from the per-row
    segment-lo values (broadcast tensor_tensor is_equal on the DVE
    engine), and matmul-accumulate [34, 16]
    blocks into a shared [34, 512] PSUM group (32 buckets per PSUM
    bank). The Act engine
    evacuates finished PSUM groups into an SBUF-resident output table
    [34, 12544] which is written back in one bulk DMA.
  - Host: concatenate the 8 disjoint slices, divide sums by
    max(count, 1), transpose to [100000, 32].
"""
import numpy as np
import ml_dtypes
import concourse.bacc as bacc
import concourse.tile as tile
import concourse.mybir as mybir
from concourse.bass_utils import run_bass_kernel_spmd

F32 = mybir.dt.float32
BF16 = mybir.dt.bfloat16
OP = mybir.AluOpType
ACT_COPY = mybir.ActivationFunctionType.Copy

N_ROWS = 4000000
D = 32
NUM_SEGMENTS = 100000
N_CORES = 8
W = 16                 # segments per bucket (one-hot width)
GB = 32                # buckets per PSUM group ([34, 512] = one 2KB bank)
E = 34                 # packed row: x(32) | 1.0 | pad
S_PAD = 100352         # 8 * 12544, >= NUM_SEGMENTS
SEG_PER_CORE = S_PAD // N_CORES      # 12544
NB = SEG_PER_CORE // W               # 196 buckets per core
CHUNK = 128            # tiles per xe load
SUB = 16               # tiles per one-hot batch instruction
DVE_SHARE = 1.0        # fraction of one-hot batches on DVE (rest on Pool;
                       # Pool TensorTensor is not walrus-legal on trn2)

_cache = {}


def _build(tiles):
    """Build the SPMD kernel for per-bucket tile counts `tiles` (len NB,
    every entry >= 1; identical across cores)."""
    total_tiles = sum(tiles)
    R = total_tiles * 128
    nc = bacc.Bacc("TRN2", target_bir_lowering=False, debug=False,
                   num_devices=N_CORES)
    xe_d = nc.dram_tensor("xe", [R * E], BF16, kind="ExternalInput")
    lo_d = nc.dram_tensor("lo", [R], BF16, kind="ExternalInput")
    iota_d = nc.dram_tensor("iota", [128, W], BF16, kind="ExternalInput")
    out_d = nc.dram_tensor("tab", [E, SEG_PER_CORE], F32,
                           kind="ExternalOutput")
    groups = [list(range(g0, min(g0 + GB, NB))) for g0 in range(0, NB, GB)]
    with tile.TileContext(nc) as tc:
        with tc.tile_pool(name="const", bufs=1) as cp, \
             tc.tile_pool(name="stream", bufs=3) as pool, \
             tc.tile_pool(name="ohp", bufs=6) as ohpool, \
             tc.tile_pool(name="psum", bufs=4, space="PSUM") as pp:
            iota = cp.tile([128, W], BF16)
            nc.sync.dma_start(out=iota[:], in_=iota_d.ap())
            ost = cp.tile([E, SEG_PER_CORE], F32)
            g_base = 0     # running tile offset
            acc = 0.0      # DVE/Pool alternation accumulator
            for g, bks in enumerate(groups):
                gw = len(bks) * W
                Tg = sum(tiles[b] for b in bks)
                ps = pp.tile([E, GB * W], F32, space="PSUM", tag="ps")
                xe_g = xe_d.ap()[g_base * 128 * E:(g_base + Tg) * 128 * E] \
                    .rearrange("(p q) -> p q", p=128)
                lo_g = lo_d.ap()[g_base * 128:(g_base + Tg) * 128] \
                    .rearrange("(p q) -> p q", p=128)
                seq = [(b, i) for b in bks for i in range(tiles[b])]
                for c0 in range(0, len(seq), CHUNK):
                    sub = seq[c0:c0 + CHUNK]
                    nsub = len(sub)
                    xe = pool.tile([128, nsub * E], BF16, tag="xe")
                    nc.sync.dma_start(out=xe[:],
                                      in_=xe_g[:, c0 * E:(c0 + nsub) * E])
                    lof = pool.tile([128, nsub], BF16, tag="lo")
                    nc.sync.dma_start(out=lof[:], in_=lo_g[:, c0:c0 + nsub])
                    # one-hots in SUB-tile batches, alternating DVE/Pool
                    for s0 in range(0, nsub, SUB):
                        ns = min(SUB, nsub - s0)
                        oh = ohpool.tile([128, SUB * W], BF16, tag="oh")
                        oh3 = oh[:].rearrange("p (t w) -> p t w", w=W)
                        acc += DVE_SHARE
                        if acc >= 1.0:
                            acc -= 1.0
                            eng = nc.vector
                        else:
                            eng = nc.gpsimd
                        eng.tensor_tensor(
                            out=oh3[:, :ns, :],
                            in0=lof[:, s0:s0 + ns].unsqueeze(-1)
                                .to_broadcast([128, ns, W]),
                            in1=iota[:].unsqueeze(1).to_broadcast([128, ns, W]),
                            op=OP.is_equal)
                        for j in range(s0, s0 + ns):
                            b, i = sub[j]
                            cw = (b - bks[0]) * W
                            nc.tensor.matmul(
                                out=ps[:, cw:cw + W],
                                lhsT=xe[:, j * E:(j + 1) * E],
                                rhs=oh[:, (j - s0) * W:(j - s0 + 1) * W],
                                start=(i == 0), stop=(i == tiles[b] - 1))
                nc.scalar.activation(out=ost[:, g * GB * W:g * GB * W + gw],
                                     in_=ps[:, :gw], func=ACT_COPY)
                g_base += Tg
            nc.sync.dma_start(out=out_d.ap(), in_=ost[:])
    nc.compile()
    return nc


def _pack_core(xb_sorted, lo_sorted, gbkt_sorted, row0, row1, core, tiles):
    """Pack one core's sorted rows into group-major [128, Tg, E] blocks.

    xb_sorted: [N, E] bf16 rows (x | 1 | pad), sorted by segment id.
    lo_sorted: [N] f32 segment-lo (idx % W) per sorted row.
    gbkt_sorted: [N] int32 global bucket id (idx // W) per sorted row.
    Rows [row0, row1) belong to this core.
    """
    xeb = []
    lob = []
    bkt = gbkt_sorted[row0:row1] - core * NB
    # bucket start offsets within the core's row range
    starts = np.searchsorted(bkt, np.arange(NB + 1))
    for b in range(NB):
        Tb = tiles[b]
        a, z = row0 + starts[b], row0 + starts[b + 1]
        nb_rows = z - a
        xx = np.zeros((Tb * 128, E), ml_dtypes.bfloat16)
        xx[:nb_rows] = xb_sorted[a:z]
        ll = np.full(Tb * 128, -1.0, ml_dtypes.bfloat16)
        ll[:nb_rows] = lo_sorted[a:z]
        xeb.append(np.ascontiguousarray(
            xx.reshape(Tb, 128, E).transpose(1, 0, 2)))
        lob.append(np.ascontiguousarray(ll.reshape(Tb, 128).T))
    xe_parts = []
    lo_parts = []
    for g0 in range(0, NB, GB):
        xe_parts.append(np.concatenate(xeb[g0:g0 + GB], axis=1).ravel())
        lo_parts.append(np.concatenate(lob[g0:g0 + GB], axis=1).ravel())
    return np.concatenate(xe_parts), np.concatenate(lo_parts)


def kernel(x, index):
    x = np.asarray(x, dtype=np.float32)
    idx = np.asarray(index).astype(np.int32)
    assert x.shape == (N_ROWS, D) and idx.shape == (N_ROWS,)

    # --- host routing: sort rows by segment, shard segment ranges ---
    order = np.argsort(idx, kind="stable")
    sidx = idx[order]
    gbkt = sidx // W                                   # global bucket id
    cnt = np.bincount(gbkt, minlength=N_CORES * NB).reshape(N_CORES, NB)
    tiles = np.maximum(((cnt + 127) // 128).max(axis=0), 1)  # max over cores
    tiles = [int(t) for t in tiles]

    key = tuple(tiles)
    if _cache.get("key") != key:
        _cache["nc"] = _build(tiles)
        _cache["key"] = key
    nc = _cache["nc"]

    xb = np.zeros((N_ROWS, E), ml_dtypes.bfloat16)
    xb[:, :D] = x[order].astype(ml_dtypes.bfloat16)
    xb[:, D] = 1.0
    lo = (sidx % W).astype(ml_dtypes.bfloat16)
    bounds = np.searchsorted(sidx, np.arange(N_CORES + 1) * SEG_PER_CORE)
    iota = np.tile(np.arange(W, dtype=np.float32), (128, 1)) \
        .astype(ml_dtypes.bfloat16)

    in_maps = []
    for c in range(N_CORES):
        xe_c, lo_c = _pack_core(xb, lo, gbkt, bounds[c], bounds[c + 1],
                                c, tiles)
        in_maps.append({"xe": xe_c, "lo": lo_c, "iota": iota})

    res = run_bass_kernel_spmd(nc, in_maps, list(range(N_CORES))).results

    tab = np.concatenate([res[c]["tab"] for c in range(N_CORES)], axis=1)
    sums = tab[:D, :NUM_SEGMENTS].T.astype(np.float64)
    counts = tab[D, :NUM_SEGMENTS].astype(np.float64)
    out = sums / np.maximum(counts, 1.0)[:, None]
    return out.astype(np.float32)


# revision 9
# speedup vs baseline: 43.5045x; 1.0709x over previous
"""Segment mean-pooling (scatter_mean) on 8 Trainium2 NeuronCores.

Strategy (segment-sharded, host-routed):
  - The output segment range [0, 100352) is sharded across the 8 cores
    (12544 segments each), so each core produces a disjoint slice of the
    output table and no all-reduce is needed.
  - The host stable-sorts rows by segment id (this is the shard/routing
    step: each row is sent to the core that owns its segment), casts x
    to bf16, appends a ones column (for counts), and packs each core's
    rows into per-bucket tiles of 128 rows. A bucket is a 16-segment
    window (W=8); tile counts per bucket position are maxed across cores so a
    single SPMD program serves all 8 cores.
  - Device, per core: stream the packed rows contiguously (no indirect
    DMA), build bf16 one-hots in 16-tile batches from the per-row
    segment-lo values (broadcast tensor_tensor is_equal on the DVE
    engine), and matmul-accumulate [34, 16]
    blocks into a shared [34, 512] PSUM group (32 buckets per PSUM
    bank). The Act engine
    evacuates finished PSUM groups into an SBUF-resident output table
    [34, 12544] which is written back in one bulk DMA.
  - Host: concatenate the 8 disjoint slices, divide sums by
    max(count, 1), transpose to [100000, 32].
"""
import numpy as np
import ml_dtypes
import concourse.bacc as bacc
import concourse.tile as tile
import concourse.mybir as mybir
from concourse.bass_utils import run_bass_kernel_spmd

F32 = mybir.dt.float32
BF16 = mybir.dt.bfloat16
OP = mybir.AluOpType
ACT_COPY = mybir.ActivationFunctionType.Copy

N_ROWS = 4000000
D = 32
NUM_SEGMENTS = 100000
N_CORES = 8
W = 8                  # segments per bucket (one-hot width)
GB = 64                # buckets per PSUM group ([33, 512] = one 2KB bank)
E = 33                 # packed row: x(32) | 1.0
S_PAD = 100352         # 8 * 12544, >= NUM_SEGMENTS
SEG_PER_CORE = S_PAD // N_CORES      # 12544
NB = SEG_PER_CORE // W               # 196 buckets per core
CHUNK = 256            # tiles per xe load
SUB = 32               # tiles per one-hot batch instruction
DVE_SHARE = 1.0        # fraction of one-hot batches on DVE (rest on Pool;
                       # Pool TensorTensor is not walrus-legal on trn2)

_cache = {}


def _build(tiles):
    """Build the SPMD kernel for per-bucket tile counts `tiles` (len NB,
    every entry >= 1; identical across cores)."""
    total_tiles = sum(tiles)
    R = total_tiles * 128
    nc = bacc.Bacc("TRN2", target_bir_lowering=False, debug=False,
                   num_devices=N_CORES)
    xe_d = nc.dram_tensor("xe", [R * E], BF16, kind="ExternalInput")
    lo_d = nc.dram_tensor("lo", [R], BF16, kind="ExternalInput")
    iota_d = nc.dram_tensor("iota", [128, W], BF16, kind="ExternalInput")
    out_d = nc.dram_tensor("tab", [E, SEG_PER_CORE], F32,
                           kind="ExternalOutput")
    groups = [list(range(g0, min(g0 + GB, NB))) for g0 in range(0, NB, GB)]
    with tile.TileContext(nc) as tc:
        with tc.tile_pool(name="const", bufs=1) as cp, \
             tc.tile_pool(name="stream", bufs=3) as pool, \
             tc.tile_pool(name="ohp", bufs=8) as ohpool, \
             tc.tile_pool(name="psum", bufs=6, space="PSUM") as pp:
            iota = cp.tile([128, W], BF16)
            nc.sync.dma_start(out=iota[:], in_=iota_d.ap())
            ost = cp.tile([E, SEG_PER_CORE], F32)
            g_base = 0     # running tile offset
            acc = 0.0      # DVE/Pool alternation accumulator
            for g, bks in enumerate(groups):
                gw = len(bks) * W
                Tg = sum(tiles[b] for b in bks)
                ps = pp.tile([E, GB * W], F32, space="PSUM", tag="ps")
                xe_g = xe_d.ap()[g_base * 128 * E:(g_base + Tg) * 128 * E] \
                    .rearrange("(p q) -> p q", p=128)
                lo_g = lo_d.ap()[g_base * 128:(g_base + Tg) * 128] \
                    .rearrange("(p q) -> p q", p=128)
                seq = [(b, i) for b in bks for i in range(tiles[b])]
                for c0 in range(0, len(seq), CHUNK):
                    sub = seq[c0:c0 + CHUNK]
                    nsub = len(sub)
                    xe = pool.tile([128, nsub * E], BF16, tag="xe")
                    nc.sync.dma_start(out=xe[:],
                                      in_=xe_g[:, c0 * E:(c0 + nsub) * E])
                    lof = pool.tile([128, nsub], BF16, tag="lo")
                    nc.sync.dma_start(out=lof[:], in_=lo_g[:, c0:c0 + nsub])
                    # one-hots in SUB-tile batches, alternating DVE/Pool
                    for s0 in range(0, nsub, SUB):
                        ns = min(SUB, nsub - s0)
                        oh = ohpool.tile([128, SUB * W], BF16, tag="oh")
                        oh3 = oh[:].rearrange("p (t w) -> p t w", w=W)
                        acc += DVE_SHARE
                        if acc >= 1.0:
                            acc -= 1.0
                            eng = nc.vector
                        else:
                            eng = nc.gpsimd
                        eng.tensor_tensor(
                            out=oh3[:, :ns, :],
                            in0=lof[:, s0:s0 + ns].unsqueeze(-1)
                                .to_broadcast([128, ns, W]),
                            in1=iota[:].unsqueeze(1).to_broadcast([128, ns, W]),
                            op=OP.is_equal)
                        for j in range(s0, s0 + ns):
                            b, i = sub[j]
                            cw = (b - bks[0]) * W
                            nc.tensor.matmul(
                                out=ps[:, cw:cw + W],
                                lhsT=xe[:, j * E:(j + 1) * E],
                                rhs=oh[:, (j - s0) * W:(j - s0 + 1) * W],
                                start=(i == 0), stop=(i == tiles[b] - 1))
                nc.scalar.activation(out=ost[:, g * GB * W:g * GB * W + gw],
                                     in_=ps[:, :gw], func=ACT_COPY)
                g_base += Tg
            nc.sync.dma_start(out=out_d.ap(), in_=ost[:])
    nc.compile()
    return nc


def _pack_core(xb_sorted, lo_sorted, gbkt_sorted, row0, row1, core, tiles):
    """Pack one core's sorted rows into group-major [128, Tg, E] blocks.

    xb_sorted: [N, E] bf16 rows (x | 1 | pad), sorted by segment id.
    lo_sorted: [N] f32 segment-lo (idx % W) per sorted row.
    gbkt_sorted: [N] int32 global bucket id (idx // W) per sorted row.
    Rows [row0, row1) belong to this core.
    """
    xeb = []
    lob = []
    bkt = gbkt_sorted[row0:row1] - core * NB
    # bucket start offsets within the core's row range
    starts = np.searchsorted(bkt, np.arange(NB + 1))
    for b in range(NB):
        Tb = tiles[b]
        a, z = row0 + starts[b], row0 + starts[b + 1]
        nb_rows = z - a
        xx = np.zeros((Tb * 128, E), ml_dtypes.bfloat16)
        xx[:nb_rows] = xb_sorted[a:z]
        ll = np.full(Tb * 128, -1.0, ml_dtypes.bfloat16)
        ll[:nb_rows] = lo_sorted[a:z]
        xeb.append(np.ascontiguousarray(
            xx.reshape(Tb, 128, E).transpose(1, 0, 2)))
        lob.append(np.ascontiguousarray(ll.reshape(Tb, 128).T))
    xe_parts = []
    lo_parts = []
    for g0 in range(0, NB, GB):
        xe_parts.append(np.concatenate(xeb[g0:g0 + GB], axis=1).ravel())
        lo_parts.append(np.concatenate(lob[g0:g0 + GB], axis=1).ravel())
    return np.concatenate(xe_parts), np.concatenate(lo_parts)


def kernel(x, index):
    x = np.asarray(x, dtype=np.float32)
    idx = np.asarray(index).astype(np.int32)
    assert x.shape == (N_ROWS, D) and idx.shape == (N_ROWS,)

    # --- host routing: sort rows by segment, shard segment ranges ---
    order = np.argsort(idx, kind="stable")
    sidx = idx[order]
    gbkt = sidx // W                                   # global bucket id
    cnt = np.bincount(gbkt, minlength=N_CORES * NB).reshape(N_CORES, NB)
    tiles = np.maximum(((cnt + 127) // 128).max(axis=0), 1)  # max over cores
    tiles = [int(t) for t in tiles]

    key = tuple(tiles)
    if _cache.get("key") != key:
        _cache["nc"] = _build(tiles)
        _cache["key"] = key
    nc = _cache["nc"]

    xb = np.zeros((N_ROWS, E), ml_dtypes.bfloat16)
    xb[:, :D] = x[order].astype(ml_dtypes.bfloat16)
    xb[:, D] = 1.0
    lo = (sidx % W).astype(ml_dtypes.bfloat16)
    bounds = np.searchsorted(sidx, np.arange(N_CORES + 1) * SEG_PER_CORE)
    iota = np.tile(np.arange(W, dtype=np.float32), (128, 1)) \
        .astype(ml_dtypes.bfloat16)

    in_maps = []
    for c in range(N_CORES):
        xe_c, lo_c = _pack_core(xb, lo, gbkt, bounds[c], bounds[c + 1],
                                c, tiles)
        in_maps.append({"xe": xe_c, "lo": lo_c, "iota": iota})

    res = run_bass_kernel_spmd(nc, in_maps, list(range(N_CORES))).results

    tab = np.concatenate([res[c]["tab"] for c in range(N_CORES)], axis=1)
    sums = tab[:D, :NUM_SEGMENTS].T.astype(np.float64)
    counts = tab[D, :NUM_SEGMENTS].astype(np.float64)
    out = sums / np.maximum(counts, 1.0)[:, None]
    return out.astype(np.float32)
